# revision 39
# baseline (speedup 1.0000x reference)
"""Causal multi-head self-attention (B=2, L=2048, E=1024, H=16) on 8 trn2 cores.

Sharding: core c handles batch b = c//4 and head-group g = c%4 (4 heads each);
the host sums the per-head-group partial outputs per batch (plus the separate
`out1` pair-1 partial for the last q-chunk) and adds w_o_b.

Per core the whole pipeline runs in a "transposed" layout so no on-device
transposes are needed:
  - host ships xT = query[b].T (contraction dim E on partitions) twice: fp8e4
    for the Q/K projections (DoubleRow matmuls) and bf16 for the V projection
    (V in fp8 fails the 2e-2 gate: attention is peaked, so V quantization
    error transfers ~directly to the output; measured 2.3e-2 in simulation,
    as does fp8 anywhere in the AV/output-projection path).
  - Q/K projections run as fp8e4 DoubleRow matmuls (2 ke-tiles per pass,
    operand layout [128, 2, n], weights host-prescaled by 512/64 into e4m3's
    normal range, descale+bias folded into one DVE tensor_scalar
    (ps*1/s)+b). Measured end-to-end error 1.28e-2 < 2e-2 gate.
  - QT is stored per-head zero-padded to K=128 (pads zeroed on GPSIMD, off
    the critical path) so the S matmuls use the full 128x128 PE array.
    (2x row-tiled K=64 S matmuls were tried and are a net LOSS: the tile
    scheduler scatters the would-be-concurrent pairs, 64<->128 tile-mode
    transitions drain the PE, and the drains read as idle to the HAM which
    re-throttles the clock 2.4->1.2GHz.)
  - V computed as [L, e_out] with a bias-driven s-column per head (emits the
    softmax row-sum from the AV matmul). Odd (hh1) heads' V columns are
    SHIFTED to stationary cols 63..127 ([s, dk]) so their AV ctx lands on
    PSUM rows 64-127 directly: both normalize muls write ctxt lane-aligned
    and no partition-shift DMA is needed. Even heads: cols 0..64 ([dk, s]).
  - S_T[k, q] exp on ACT (no max-sub: |scores| <= ~5) batched per kt-pair
    [128, 2, 512]; causal mask applied multiplicatively on diagonal tiles
    (DVE). Diagonal tiles use per-kt exact-causal query offsets (tile
    r=kt-4qc computes q in [128r:512) only); untouched av PSUM stays
    has_written=0 until a later wider kt's AV overwrites it. The exp runs
    at the PAIR's width (junk PSUM in the narrow kt's slice exps harmlessly;
    per-kt exp ops would add 64 ACT issue overheads).
  - ctxT = ctxT_unnorm * (1/s): ONE PE matmul per (pair, chunk) broadcasts
    BOTH heads' denominators via a dual-row selector (col m reads rrow row
    64 for m<64 -- hh0's s -- and row 63 for m>=64 -- hh1's s), then one
    reciprocal_approx_fast [128, 512] serves both muls. hh1's s row sits at
    av row 63; compute engines need 32-aligned partition bases, so it is
    copied as the tail of the [32:64) block (rows 32-62 are exact zeros:
    v_sb odd cols 32-62 are memset once).
  - y_partial = ctxT.T @ woT, emitted bf16 (host accumulates in fp32).

Scheduling (the tile scheduler is a priority heap over a dependency graph;
engine queues execute IN ORDER, so a blocked instruction blocks everything
behind it on that engine):
  - Each attention kt-pair quantum yields TWICE (pre-AV and post-AV) so
    projection/output-projection fills slot in while the ACT exp drains.
  - Do NOT put Identity-activation bias ops on ACT, even where the queue
    looks idle: two chunk-0 bias ops measured +30us (ACT table-set switch
    between Identity and Exp costs ~2.7us per reload, and the scheduler
    re-orders around them). Copy-func ops (scalar.copy) in the exp-free
    tail are fine.
  - Startup: Q/K pt0 project first, then attention (0,0) starts while the
    remaining chunk-0 groups fill the exp-wait gaps (V groups in DESCENDING
    l-tile order ahead of the AVs that consume them).
  - DMAs: per-queue bandwidth is only ~55-85GB/s and queues process their
    DMAs serially, so transfers are few and full-tensor (2-8KB contiguous
    lines), spread over the sync/scalar/gpsimd queues in consumption order;
    chunk-1/2 x/x8 are prefetched from the startup block (in the fills they
    sit behind ~1MB of queued weights and stall the next era by ~2-6us).
  - Dummy matmuls over uninitialized SBUF warm the HAM clock gate during
    the input-DMA wait, bridge the PE-sparse window before the V
    projections, and (emitted last, so scheduled only when nothing else is
    ready) keep the clock warm across the final normalize chain.
  - Unit order ends with qc=2: outproj(3) weaves into the last eras, and
    outproj(2) is pair-split - pair-0 runs as fills inside the last unit,
    only pair-1 (8 matmuls + copies + DMA to `out1`, summed on host) trails
    the exp stream.
"""

import numpy as np

import concourse.bass as bass
import concourse.mybir as mybir
import concourse.tile as tile
from concourse import bacc
from concourse.bass_utils import run_bass_kernel_spmd

F32 = mybir.dt.float32
BF16 = mybir.dt.bfloat16
F8 = mybir.dt.float8e4
AF = mybir.ActivationFunctionType
ALU = mybir.AluOpType
DR = mybir.MatmulPerfMode.DoubleRow

B, L, E, H, DK = 2, 2048, 1024, 16, 64
NCORES, GROUPS = 8, 4
HL = 256          # local head dims per core (4 heads x 64)
HLV = 260         # V projection width: 4 heads x (64 + ones column)
QC = 512          # q-chunk (matmul free dim)
NQC = L // QC     # 4
NKT = L // 128    # 16 k-tiles
NE = E // 128     # 8 contraction tiles for projections
NEP = NE // 2     # 4 DoubleRow ke-pairs
SCALE = 1.0 / np.sqrt(DK)
WQS = 512.0       # fp8 prescale for wq (and bq)
WKS = 64.0        # fp8 prescale for wk (and bk)


def _emit(nc, tc, t):
    import contextlib
    from collections import deque

    ctx = contextlib.ExitStack()
    with ctx:
        persist = ctx.enter_context(tc.tile_pool(name="persist", bufs=1))

        qt = persist.tile([128, 4, L], BF16, tag="qt")
        kt_sb = persist.tile([128, 2, L], BF16, tag="kt")
        v_sb = persist.tile([128, NKT, 2, 2, 128], BF16, tag="v")
        ctxt = persist.tile([128, 2, L], BF16, tag="ctxt")
        mask = persist.tile([128, 896], BF16, tag="mask")
        wq = persist.tile([128, NE, HL], F8, tag="wq")
        wk = persist.tile([128, NE, HL], F8, tag="wk")
        wv = persist.tile([128, NE, HLV], BF16, tag="wv")
        wo = persist.tile([128, 2, E], BF16, tag="wo")
        bqk = persist.tile([128, 4], F32, tag="bqk")
        bv = persist.tile([128, HLV], F32, tag="bv")
        ones = persist.tile([128, 128], BF16, tag="ones")
        rrows = persist.tile([128, 4, QC], BF16, tag="rrows")

        ypool = ctx.enter_context(tc.tile_pool(name="ypool", bufs=6))
        phase = contextlib.ExitStack()
        x8pool = phase.enter_context(tc.tile_pool(name="x8pool", bufs=3))
        xbpool = phase.enter_context(tc.tile_pool(name="xbpool", bufs=3))
        psproj = phase.enter_context(
            tc.tile_pool(name="psproj", bufs=2, space="PSUM")
        )
        ppool = phase.enter_context(tc.tile_pool(name="ppool", bufs=8))
        pss = phase.enter_context(tc.tile_pool(name="pss", bufs=2, space="PSUM"))
        psav = phase.enter_context(tc.tile_pool(name="psav", bufs=2, space="PSUM"))
        npool = phase.enter_context(tc.tile_pool(name="npool", bufs=4))

        # wq/wk and the first x chunk gate the first matmuls; xtb/wv gate the
        # first V groups (~13us in). Each queue processes its DMAs SERIALLY
        # at ~55-85GB/s, so the chunk-0-era inputs (~2.6MB) are cut into
        # ~256KB pieces spread over ALL FIVE engine queues, ordered per queue
        # by consumption time. (The tensor queue's descriptors run before the
        # warm matmuls -- emission order -- costing ~0.6us of warm-up delay.)
        # Only sync/scalar/gpsimd can issue DMAs; each is a FIFO ring whose
        # transfers spray across all 16 SDMA engines, and the active rings
        # share ~358GB/s round-robin. So: per ring, strictly consumption
        # order; chunk-1/2 prefetches go LAST on the same rings (FIFO means
        # they cannot steal bandwidth from chunk-0 pieces, unlike the old
        # scheme that interleaved them early).
        xt8_0 = x8pool.tile([128, NE, QC], F8, tag="x8", name="x8_0")
        xtb_0 = xbpool.tile([128, 4, NE, 128], BF16, tag="xb", name="xb_0")
        # ring plan (each ring FIFO, ~2us fixed cost per DMA): per ring the
        # chunk-0-critical pieces first in consumption order, prefetches
        # strictly after. smalls carries bqk early (the Q-bias DVE ops gate
        # K pt0's PSUM reuse through the pinned warm_ps buf -- a late bqk
        # once stalled the PE 8.4us).
        # bqk FIRST and tiny: the Q-bias DVE ops gate K pt0's PSUM reuse
        # (psproj WAR through the pinned warm_ps buf) -- a late bqk once
        # stalled the PE 8.4us
        nc.sync.dma_start(out=bqk, in_=t["bqk"][:])
        nc.scalar.dma_start(out=xt8_0[:, 0:4, :], in_=t["x8"][0, :, 0:4, :])
        nc.gpsimd.dma_start(out=xt8_0[:, 4:8, :], in_=t["x8"][0, :, 4:8, :])
        nc.sync.dma_start(out=wq, in_=t["wq"][:])
        nc.sync.dma_start(out=wk, in_=t["wk"][:])
        # V inputs (xb is laid out per l-tile; V groups consume lt DESCENDING
        # so lt3 ships first; wv ke-halves land in accumulation order)
        nc.scalar.dma_start(out=xtb_0[:, 3], in_=t["xb"][0, :, 3])
        nc.gpsimd.dma_start(out=xtb_0[:, 2], in_=t["xb"][0, :, 2])
        nc.sync.dma_start(out=wv[:, 0:4, :], in_=t["wv"][:, 0:4, :])
        nc.scalar.dma_start(out=wv[:, 4:8, :], in_=t["wv"][:, 4:8, :])
        nc.gpsimd.dma_start(out=xtb_0[:, 1], in_=t["xb"][0, :, 1])
        nc.scalar.dma_start(out=xtb_0[:, 0], in_=t["xb"][0, :, 0])
        nc.sync.dma_start(out=mask, in_=t["mask"][:])
        nc.sync.dma_start(out=bv, in_=t["bv"][:])
        nc.sync.dma_start(out=ones, in_=t["ones"][:])
        nc.sync.dma_start(out=wo, in_=t["wo"][:])
        # chunk 1/2 prefetch (consumed from ~45us / ~85us): behind chunk-0
        # in ring order, still issued from startup so they never queue
        # behind mid-run output DMAs
        xt8_1 = x8pool.tile([128, NE, QC], F8, tag="x8", name="x8_1")
        xtb_1 = xbpool.tile([128, 4, NE, 128], BF16, tag="xb", name="xb_1")
        nc.sync.dma_start(out=xt8_1, in_=t["x8"][1])
        nc.gpsimd.dma_start(out=xtb_1[:, 2:4], in_=t["xb"][1, :, 2:4])
        nc.scalar.dma_start(out=xtb_1[:, 0:2], in_=t["xb"][1, :, 0:2])
        xt8_2 = x8pool.tile([128, NE, QC], F8, tag="x8", name="x8_2")
        xtb_2 = xbpool.tile([128, 4, NE, 128], BF16, tag="xb", name="xb_2")
        nc.gpsimd.dma_start(out=xt8_2, in_=t["x8"][2])
        nc.sync.dma_start(out=xtb_2[:, 2:4], in_=t["xb"][2, :, 2:4])
        nc.scalar.dma_start(out=xtb_2[:, 0:2], in_=t["xb"][2, :, 0:2])

        # HAM warm-up: the PE clock starts gated at 1.2GHz and only reaches
        # 2.4GHz after ~3.4us of sustained matmul activity -- which used to
        # happen DURING the first (DMA-paced) projection matmuls. Burn the
        # input-DMA wait window on dummy matmuls over uninitialized SBUF
        # (ctxt, first written much later; scratch PSUM never read) so the
        # real matmuls start already warm.
        warm_ps = psproj.tile([128, QC], F32, tag="ps", name="warm")
        for _ in range(16):
            nc.tensor.matmul(
                warm_ps,
                lhsT=ctxt[:, 1, 0:128],
                rhs=ctxt[:, 0, 0:QC],
                start=True,
                stop=True,
            )

        # qt holds each head zero-padded to K=128 (head 2i in rows 0-63 of
        # slot 2i, head 2i+1 in rows 64-127 of slot 2i+1) so the S matmuls
        # use the full 128x128 PE array (half-array matmuls make the PE HAM
        # activity monitor throttle the clock 2.4 -> 1.2 GHz). The pad
        # halves are zeroed ONCE on the DVE, which is idle until the first
        # mask multiply (~16us): the gpsimd queue must stay short here so
        # the (gpsimd) Q/K bias ops that gate the first S matmuls run on
        # time.
        for sl4 in range(4):
            psl = slice(DK, 128) if sl4 % 2 == 0 else slice(0, DK)
            nc.gpsimd.memset(qt[psl, sl4, :], 0.0)
        # rrows stays zero except rows 63/64 (written per normalize): the
        # broadcast matmul's selector zeros then annihilate the other rows
        nc.gpsimd.memset(rrows, 0.0)
        # odd-head V stationary cols 32-62 must be 0.0 (not SBUF garbage):
        # av rows 32-62 are copied alongside the s row (see normalize) and
        # 0*NaN would poison the selector matmul
        nc.gpsimd.memset(v_sb[:, :, :, 1, 32:DK], 0.0)

        def proj_group(qc, xt8, xtb, gi):
            """One projection accumulation group.
            gi 0-1: Q head-pair gi (fp8 DoubleRow); gi 2-3: K head-pair gi-2
            (fp8 DoubleRow); gi 4-7: V l-tile gi-4 (bf16)."""
            qsl = slice(qc * QC, (qc + 1) * QC)
            if gi < 4:
                w_sb, bo0, is_q, pt, ds = (
                    (wq, 0, True, gi, 1.0 / WQS)
                    if gi < 2
                    else (wk, 2, False, gi - 2, 1.0 / WKS)
                )
                b_sb = bqk[:, bo0 : bo0 + 2]
                ps = psproj.tile([128, QC], F32, tag="ps", name=f"ps{qc}{gi}")
                for kp in range(NEP):
                    nc.tensor.matmul(
                        ps,
                        lhsT=w_sb[:, 2 * kp : 2 * kp + 2, pt * 128 : (pt + 1) * 128],
                        rhs=xt8[:, 2 * kp : 2 * kp + 2, :],
                        start=(kp == 0),
                        stop=(kp == NEP - 1),
                        perf_mode=DR,
                    )
                # descale+bias on GPSIMD: out = (ps * ds) + b. On DVE these
                # 24 ops (750ns each) backed up the strict-FIFO vector queue
                # at era transitions and the PE's next projection group
                # stalled ~2us on the psproj-buf WAR against the bias READ.
                # GPSIMD is ~85% idle and drains them promptly. (ACT was
                # also tried: the extra ops clog the exp stream, +15us.)
                if is_q:
                    nc.vector.tensor_scalar(
                        out=qt[0:DK, 2 * pt, qsl],
                        in0=ps[0:DK, :],
                        scalar1=ds,
                        scalar2=b_sb[0:DK, pt : pt + 1],
                        op0=ALU.mult,
                        op1=ALU.add,
                    )
                    nc.vector.tensor_scalar(
                        out=qt[DK:128, 2 * pt + 1, qsl],
                        in0=ps[DK:128, :],
                        scalar1=ds,
                        scalar2=b_sb[DK:128, pt : pt + 1],
                        op0=ALU.mult,
                        op1=ALU.add,
                    )
                else:
                    nc.vector.tensor_scalar(
                        out=kt_sb[:, pt, qsl],
                        in0=ps,
                        scalar1=ds,
                        scalar2=b_sb[:, pt : pt + 1],
                        op0=ALU.mult,
                        op1=ALU.add,
                    )
            else:
                lt4 = gi - 4
                lt = qc * 4 + lt4
                ps = psproj.tile([128, QC], F32, tag="ps", name=f"psv{lt}")
                psv = ps[:, 0:HLV]
                for ke in range(NE):
                    nc.tensor.matmul(
                        psv,
                        lhsT=xtb[:, lt4, ke, :],
                        rhs=wv[:, ke, :],
                        start=(ke == 0),
                        stop=(ke == NE - 1),
                    )
                # psv/bv column order is [h0 h2 | h1 h3] (hh-major): even-slot
                # heads write cols 0:65 ([dk, s]); odd-slot heads write cols
                # 63:128 ([s, dk]) so their AV ctx lands on PSUM rows 64-127.
                nc.vector.tensor_add(
                    out=v_sb[:, lt, 0:2, 0, 0 : DK + 1],
                    in0=psv[:, 0 : 2 * (DK + 1)].rearrange(
                        "p (h d) -> p h d", d=DK + 1
                    ),
                    in1=bv[:, 0 : 2 * (DK + 1)].rearrange(
                        "p (h d) -> p h d", d=DK + 1
                    ),
                )
                nc.vector.tensor_add(
                    out=v_sb[:, lt, 0:2, 1, DK - 1 : 128],
                    in0=psv[:, 2 * (DK + 1) : 4 * (DK + 1)].rearrange(
                        "p (h d) -> p h d", d=DK + 1
                    ),
                    in1=bv[:, 2 * (DK + 1) : 4 * (DK + 1)].rearrange(
                        "p (h d) -> p h d", d=DK + 1
                    ),
                )

        def proj_fills(qc):
            box = {}

            def mk(gi):
                def f():
                    if gi == 0:
                        if qc == 1:
                            box["x"] = (xt8_1, xtb_1)
                        elif qc == 2:
                            box["x"] = (xt8_2, xtb_2)
                        else:
                            xt8 = x8pool.tile(
                                [128, NE, QC], F8, tag="x8", name=f"x8_{qc}"
                            )
                            xtb = xbpool.tile(
                                [128, 4, NE, 128], BF16, tag="xb", name=f"xb_{qc}"
                            )
                            nc.sync.dma_start(out=xt8, in_=t["x8"][qc])
                            nc.gpsimd.dma_start(
                                out=xtb[:, 2:4], in_=t["xb"][qc, :, 2:4]
                            )
                            nc.sync.dma_start(
                                out=xtb[:, 0:2], in_=t["xb"][qc, :, 0:2]
                            )
                            box["x"] = (xt8, xtb)
                    proj_group(qc, *box["x"], gi)

                return f

            return [mk(gi) for gi in range(8)]

        def outproj_fills(qc, alternate=False):
            ybox = {}
            pbox = {}

            def mk(lt, ec, pair, idx):
                def f():
                    lsl = slice(lt * 128, (lt + 1) * 128)
                    esl = slice(ec * QC, (ec + 1) * QC)
                    if pair == 0:
                        pbox[(lt, ec)] = psproj.tile(
                            [128, QC], F32, tag="ps", name=f"y{lt}{ec}"
                        )
                    ps = pbox[(lt, ec)]
                    nc.tensor.matmul(
                        ps,
                        lhsT=ctxt[:, pair, lsl],
                        rhs=wo[:, pair, esl],
                        start=(pair == 0),
                        stop=(pair == 1),
                    )
                    if pair == 0:
                        return
                    # both ec halves share one [128, 2*QC] tile: ONE 2KB-line
                    # DMA per l-tile instead of two 1KB-line ones (out-DMAs
                    # share queues with the x-chunk prefetches)
                    if ec == 0:
                        ybox[lt] = ypool.tile(
                            [128, 2, QC], BF16, tag="ysb", name="ysb"
                        )
                    ysb = ybox[lt]
                    if alternate and idx % 2 == 0:
                        nc.scalar.copy(out=ysb[:, ec, :], in_=ps)
                    else:
                        nc.vector.tensor_copy(out=ysb[:, ec, :], in_=ps)
                    if ec == 1:
                        eng = nc.sync if lt % 2 == 0 else nc.gpsimd
                        eng.dma_start(out=t["out"][lsl, :], in_=ysb)

                return f

            return [
                mk(lt, ec, pair, 2 * (lt - qc * 4) + ec)
                for lt in range(qc * 4, qc * 4 + 4)
                for ec in range(2)
                for pair in range(2)
            ]

        # outproj for the LAST unit's q-range, split by pair: the pair-0
        # matmuls run as fills DURING the final attention unit (keeping the
        # PE warm across the exp drain) and DMA to `out`; the pair-1 halves
        # trail the last exp, DMA to `out1`, and the HOST adds the two
        # partials (it already sums 4 cores' partials per batch anyway).
        def outproj_p0_fills(qc):
            def mk(lt, ec, idx):
                def f():
                    lsl = slice(lt * 128, (lt + 1) * 128)
                    esl = slice(ec * QC, (ec + 1) * QC)
                    ps = psproj.tile([128, QC], F32, tag="ps", name=f"z{lt}{ec}")
                    nc.tensor.matmul(
                        ps, lhsT=ctxt[:, 0, lsl], rhs=wo[:, 0, esl],
                        start=True, stop=True,
                    )
                    ysb = ypool.tile([128, QC], BF16, tag="ysb0", name="ysb0")
                    nc.vector.tensor_copy(out=ysb, in_=ps)
                    deng = nc.sync if idx % 2 == 0 else nc.gpsimd
                    deng.dma_start(out=t["out"][lsl, esl], in_=ysb)

                return f

            return [
                mk(lt, ec, 2 * (lt - qc * 4) + ec)
                for lt in range(qc * 4, qc * 4 + 4)
                for ec in range(2)
            ]

        def outproj_p1_finish(qc):
            # DMA per (lt, ec) half the moment its copy lands, spread over
            # four queues: the trailing out1 transfers (1MB total) pace the
            # kernel tail, so smaller earlier pieces finish sooner.
            for idx, (lt, ec) in enumerate(
                (lt, ec)
                for lt in range(qc * 4, qc * 4 + 4)
                for ec in range(2)
            ):
                lsl = slice(lt * 128, (lt + 1) * 128)
                esl = slice(ec * QC, (ec + 1) * QC)
                rsl = slice((lt - qc * 4) * 128, (lt - qc * 4 + 1) * 128)
                ps = psproj.tile([128, QC], F32, tag="ps", name=f"w{lt}{ec}")
                nc.tensor.matmul(
                    ps, lhsT=ctxt[:, 1, lsl], rhs=wo[:, 1, esl],
                    start=True, stop=True,
                )
                ysb = ypool.tile([128, QC], BF16, tag="ysb1", name="ysb1")
                if idx % 2 == 0:
                    nc.scalar.copy(out=ysb, in_=ps)
                else:
                    nc.vector.tensor_copy(out=ysb, in_=ps)
                deng = (nc.sync, nc.gpsimd, nc.scalar)[idx % 3]
                deng.dma_start(out=t["out1"][rsl, esl], in_=ysb)

        nrm_ctr = [0]

        def gen_attn(pair, qc, hh_order=(0, 1)):
            """Yields TWICE per kt-pair quantum: after S+exp+mask emission
            (pre-AV: the AVs wait on the ACT exp, so fills here keep the
            in-order PE queue busy) and after the AVs."""
            nkt = 4 * qc + 4
            qsl = slice(qc * QC, (qc + 1) * QC)
            avs = [
                psav.tile([128, QC], F32, tag="av", name=f"av{pair}{qc}{i}")
                for i in range(2)
            ]
            # kt descends in pairs: diagonal (partially masked) tiles FIRST so
            # AV matmuls' single LDWEIGHTS wait slot works out (diagonal AVs
            # wait on the DVE mask-multiply whose sem value subsumes all older
            # DVE writes; non-diagonal AVs wait only on the exp).
            for kt_hi in range(nkt - 1, 0, -2):
                kts = (kt_hi, kt_hi - 1)
                # per-kt exact-causal query offset: diagonal tile r=kt-4qc
                # only attends q >= 128r within the chunk, so S/mask/AV run
                # on [128r:512) only. The untouched av PSUM region stays
                # has_written=0 until a later (wider) kt's AV overwrites it
                # (per-element has_written semantics). The exp still runs at
                # the PAIR's width (one ACT op per hh; the narrow kt's
                # [qo:qo_i) slice exps junk PSUM that nothing reads --
                # splitting the exp per kt would add 64 ACT issue overheads).
                qos = [max(kt - 4 * qc, 0) * 128 for kt in kts]
                qo = qos[1]
                w = QC - qo
                sps = [
                    pss.tile([128, 2, QC], F32, tag="s", name=f"s{hh}")
                    for hh in range(2)
                ]
                for hh in range(2):
                    h = pair * 2 + hh
                    for i, kt in enumerate(kts):
                        nc.tensor.matmul(
                            sps[hh][:, i, qos[i] : QC],
                            lhsT=kt_sb[:, pair, kt * 128 : (kt + 1) * 128],
                            rhs=qt[:, h, qc * QC + qos[i] : (qc + 1) * QC],
                            start=True,
                            stop=True,
                        )
                pms = []
                for hh in range(2):
                    p_e = ppool.tile([128, 2, QC], BF16, tag="p", name="p_e")
                    nc.scalar.activation(
                        out=p_e[:, :, qo:QC], in_=sps[hh][:, :, qo:QC], func=AF.Exp
                    )
                    pm_h = []
                    for i, kt in enumerate(kts):
                        r = kt - 4 * qc
                        wi = QC - qos[i]
                        if r >= 0:
                            # mask col math: tile r, query q' in [128r:512)
                            # maps to mask cols (3-r)*128 + q' = [384:384+wi)
                            p_m = ppool.tile(
                                [128, QC], BF16, tag="pm", name="p_m"
                            )
                            nc.vector.tensor_mul(
                                out=p_m[:, qos[i] : QC],
                                in0=p_e[:, i, qos[i] : QC],
                                in1=mask[:, 384 : 384 + wi],
                            )
                            pm_h.append(p_m[:, qos[i] : QC])
                        else:
                            pm_h.append(p_e[:, i, qos[i] : QC])
                    pms.append(pm_h)
                yield
                for hh in range(2):
                    for i, kt in enumerate(kts):
                        nc.tensor.matmul(
                            avs[hh][:, qos[i] : QC],
                            lhsT=v_sb[:, kt, pair, hh, :],
                            rhs=pms[hh][i],
                            start=(kt == nkt - 1),
                            stop=(kt == 0),
                        )
                yield
            # normalize: ctxT = ctx_unnorm / s. The V layout puts hh0's ctx
            # on av0 rows 0-63 (s at row 64) and hh1's ctx on av1 rows
            # 64-127 (s at row 63), so ONE selector matmul broadcasts both
            # denominators (sel col m reads row 64 for m<64, row 63 for
            # m>=64), one reciprocal serves both muls, and both muls write
            # ctxt lane-aligned -- no partition-shift DMA.
            rrow = rrows[:, nrm_ctr[0] % 4, :]
            nrm_ctr[0] += 1
            nc.vector.tensor_copy(
                out=rrow[DK : DK + 1, :], in_=avs[0][DK : DK + 1, :]
            )
            # compute engines need 32-aligned partition bases: copy hh1's s
            # (row 63) as the tail of the [32:64) block. av rows 32-62 are
            # exact 0.0 (v_sb odd cols 32-62 are memset once at startup), so
            # the copy keeps rrow's selector-dead rows zero.
            nc.vector.tensor_copy(
                out=rrow[DK - 32 : DK, :], in_=avs[1][DK - 32 : DK, :]
            )
            sbc = psproj.tile([128, QC], F32, tag="ps", name="sbc")
            nc.tensor.matmul(sbc, lhsT=ones, rhs=rrow, start=True, stop=True)
            rbc = npool.tile([128, QC], F32, tag="rbc", name="rbc")
            nc.vector.reciprocal_approx_fast(out=rbc, in_=sbc)
            nc.vector.tensor_mul(
                out=ctxt[0:DK, pair, qsl], in0=avs[0][0:DK, :], in1=rbc[0:DK, :]
            )
            nc.vector.tensor_mul(
                out=ctxt[DK:128, pair, qsl],
                in0=avs[1][DK:128, :],
                in1=rbc[DK:128, :],
            )

        # --- startup weave: Q/K pt0 only, then attention (0,0) starts while
        # the remaining chunk-0 groups fill the exp-wait gaps. V groups emit
        # in DESCENDING l-tile order ahead of the AVs that consume them. ---
        g0 = lambda gi: proj_group(0, xt8_0, xtb_0, gi)
        proj_group(0, xt8_0, xtb_0, 0)      # Q pt0
        # no-dep dummies BETWEEN Q pt0 and K pt0: K pt0's first matmul has a
        # PSUM WAR on the Q-bias DVE reads (psproj buf reuse), so the PE
        # otherwise idles here long enough for the HAM MID window to
        # re-throttle the clock
        for _ in range(8):
            nc.tensor.matmul(
                warm_ps, lhsT=ctxt[:, 1, 0:128], rhs=ctxt[:, 0, 0:QC],
                start=True, stop=True,
            )
        proj_group(0, xt8_0, xtb_0, 2)      # K pt0
        it = gen_attn(0, 0)
        next(it)                            # S+exp (kt 3,2)
        for _ in range(14):
            nc.tensor.matmul(
                warm_ps, lhsT=ctxt[:, 1, 0:128], rhs=ctxt[:, 0, 0:QC],
                start=True, stop=True,
            )
        g0(7)                               # V lt3
        g0(6)                               # V lt2
        next(it)                            # AV (kt 3,2)
        g0(1)                               # Q pt1
        next(it)                            # S+exp (kt 1,0)
        g0(5)                               # V lt1
        g0(4)                               # V lt0
        next(it)                            # AV (kt 1,0)
        g0(3)                               # K pt1
        for _ in it:                        # normalize (0,0)
            pass

        # Unit order ends with qc=2 (not qc=3) so outproj(3) can weave INTO
        # the last eras and only outproj(2)'s pair-1 half (8 small matmuls +
        # adds) trails the exp stream: the tail after the last exp was 18us
        # with qc=3 last and monolithic outproj groups.
        eras = [
            ([(1, 0)], proj_fills(1), 2),
            ([(0, 1), (1, 1)], proj_fills(2) + proj_fills(3), 1),
            (
                [(0, 3), (1, 3)],
                outproj_fills(0) + outproj_fills(1),
                1,
            ),
            ([(0, 2)], outproj_fills(3), 1),
            # pad the last era's fill queue so the pair-0 output groups land
            # LATE in the unit -- they bridge the PE across the final
            # normalize chain so the HAM stays warm for the pair-1 matmuls
            ([(1, 2)], [None] * 4 + outproj_p0_fills(2), 1),
        ]
        for units, fills, k in eras:
            fills = deque(fills)
            for pair, qc in units:
                hh_order = (1, 0) if (pair, qc) == (1, 2) else (0, 1)
                for _ in gen_attn(pair, qc, hh_order):
                    for _ in range(k):
                        if fills:
                            f = fills.popleft()
                            if f is not None:
                                f()
            while fills:
                f = fills.popleft()
                if f is not None:
                    f()
        for i in range(6):
            wps = psproj.tile([128, QC], F32, tag="ps", name="warmtail")
            nc.tensor.matmul(
                wps, lhsT=ctxt[:, 1, 0:128], rhs=ctxt[:, 0, 0:QC],
                start=True, stop=True,
            )
        outproj_p1_finish(2)
        phase.close()


def build_nc():
    nc = bacc.Bacc("TRN2", target_bir_lowering=False)
    t = {
        "x8": nc.dram_tensor("x8", [NQC, 128, NE, QC], F8, kind="ExternalInput")[:],
        "xb": nc.dram_tensor(
            "xb", [NQC, 128, 4, NE, 128], BF16, kind="ExternalInput"
        )[:],
        "wq": nc.dram_tensor("wq", [128, NE, HL], F8, kind="ExternalInput")[:],
        "wk": nc.dram_tensor("wk", [128, NE, HL], F8, kind="ExternalInput")[:],
        "wv": nc.dram_tensor("wv", [128, NE, HLV], BF16, kind="ExternalInput")[:],
        "wo": nc.dram_tensor("wo", [128, 2, E], BF16, kind="ExternalInput")[:],
        "bqk": nc.dram_tensor("bqk", [128, 4], F32, kind="ExternalInput")[:],
        "bv": nc.dram_tensor("bv", [128, HLV], F32, kind="ExternalInput")[:],
        "mask": nc.dram_tensor("mask", [128, 896], BF16, kind="ExternalInput")[:],
        "ones": nc.dram_tensor("ones", [128, 128], BF16, kind="ExternalInput")[:],
        "out": nc.dram_tensor("out", [L, E], BF16, kind="ExternalOutput")[:],
        "out1": nc.dram_tensor("out1", [QC, E], BF16, kind="ExternalOutput")[:],
    }
    with tile.TileContext(nc) as tc:
        _emit(nc, tc, t)
    nc.compile()
    return nc


def _dev_layout(arr, kind, dt=None):
    """Host -> device data layouts (see build_nc tensor shapes)."""
    import ml_dtypes

    if dt is None:
        dt = ml_dtypes.bfloat16
    a = np.ascontiguousarray(arr, dtype=np.float32)
    if kind == "x":  # [1024, 2048] (already transposed) -> [NQC, 128, NE, QC]
        return np.ascontiguousarray(
            a.reshape(NE, 128, NQC, QC).transpose(2, 1, 0, 3)
        ).astype(dt)
    if kind == "xlt":  # [1024, 2048] (xT) -> [NQC, 128, 4, NE, 128] (per-lt)
        return np.ascontiguousarray(
            a.reshape(NE, 128, NQC, 4, 128).transpose(2, 1, 3, 0, 4)
        ).astype(dt)
    if kind == "w3":  # [1024, W] (wT) -> [128, NE, W]
        w = a.shape[1]
        return np.ascontiguousarray(
            a.reshape(NE, 128, w).transpose(1, 0, 2)
        ).astype(dt)
    if kind == "wo":  # [HL, 1024] (woT) -> [128, 2, E]
        return np.ascontiguousarray(
            a.reshape(2, 128, E).transpose(1, 0, 2)
        ).astype(dt)
    if kind == "b":  # [HL] -> [128, 2]
        return np.ascontiguousarray(a.reshape(2, 128).T)
    raise ValueError(kind)


def _augment_v(vT):
    """[R, 256] -> [R, 260] in AV-stationary column order [h0 h2 | h1 h3]:
    even-slot (hh0) heads get [dk(64), s-col], odd-slot (hh1) heads get
    [s-col, dk(64)]. The s-col (0-weights, 1.0-bias) makes the AV matmul
    emit the softmax row-sum; the hh1 shift lands its ctx on PSUM rows
    64-127 (s at row 63) so the normalize muls write ctxt lane-aligned."""
    r = vT.shape[0]
    v4 = vT.reshape(r, 4, DK)
    pad_val = 1.0 if r == 1 else 0.0
    pad = np.full((r, 1), pad_val, np.float32)
    blocks = [
        np.concatenate([v4[:, 0], pad], axis=1),  # head0 (p0,hh0): dk, s
        np.concatenate([v4[:, 2], pad], axis=1),  # head2 (p1,hh0): dk, s
        np.concatenate([pad, v4[:, 1]], axis=1),  # head1 (p0,hh1): s, dk
        np.concatenate([pad, v4[:, 3]], axis=1),  # head3 (p1,hh1): s, dk
    ]
    return np.ascontiguousarray(np.concatenate(blocks, axis=1))


def _sel_ones():
    """Dual-row broadcast selector: out rows 0-63 read rrow row 64 (hh0's
    s), rows 64-127 read rrow row 63 (hh1's s)."""
    import ml_dtypes

    a = np.zeros((128, 128), ml_dtypes.bfloat16)
    a[DK, 0:DK] = 1.0
    a[DK - 1, DK:128] = 1.0
    return a


def make_in_maps(query, w_q_w, w_q_b, w_k_w, w_k_b, w_v_w, w_v_b, w_o_w, w_o_b):
    import ml_dtypes

    F8NP = ml_dtypes.float8_e4m3

    mask = (
        np.arange(896, dtype=np.int64)[None, :]
        >= (np.arange(128, dtype=np.int64)[:, None] + 384)
    ).astype(ml_dtypes.bfloat16)
    x8_dev = [
        _dev_layout(np.asarray(query[b], np.float32).T, "x", F8NP)
        for b in range(B)
    ]
    xb_dev = [
        _dev_layout(np.asarray(query[b], np.float32).T, "xlt") for b in range(B)
    ]
    in_maps = []
    for c in range(NCORES):
        b, g = divmod(c, GROUPS)
        rows = slice(g * HL, (g + 1) * HL)
        in_maps.append(
            {
                "x8": x8_dev[b],
                "xb": xb_dev[b],
                "wq": _dev_layout(
                    np.asarray(w_q_w)[rows, :].T * (SCALE * WQS), "w3", F8NP
                ),
                "wk": _dev_layout(np.asarray(w_k_w)[rows, :].T * WKS, "w3", F8NP),
                "wv": _dev_layout(_augment_v(np.asarray(w_v_w)[rows, :].T), "w3"),
                "wo": _dev_layout(np.asarray(w_o_w)[:, rows].T, "wo"),
                "bqk": np.concatenate(
                    [
                        _dev_layout(np.asarray(w_q_b)[rows] * SCALE, "b"),
                        _dev_layout(np.asarray(w_k_b)[rows], "b"),
                    ],
                    axis=1,
                ),
                "bv": np.ascontiguousarray(
                    np.broadcast_to(
                        _augment_v(np.asarray(w_v_b, np.float32)[rows][None, :])[0],
                        (128, HLV),
                    )
                ),
                "mask": mask,
                "ones": _sel_ones(),
            }
        )
    return in_maps


_NC_CACHE = {}


def kernel(trace=False, **inputs):
    if "nc" not in _NC_CACHE:
        _NC_CACHE["nc"] = build_nc()
    nc = _NC_CACHE["nc"]
    in_maps = make_in_maps(**inputs)
    res = run_bass_kernel_spmd(
        nc,
        in_maps,
        core_ids=list(range(NCORES)),
        trace=trace,
        trace_cores=[0] if trace else None,
    )
    w_o_b = np.asarray(inputs["w_o_b"], np.float32)
    out = np.zeros((B, L, E), dtype=np.float32)
    for c in range(NCORES):
        b = c // GROUPS
        out[b] += res.results[c]["out"].astype(np.float32)
        out[b, 2 * QC : 3 * QC] += res.results[c]["out1"].astype(np.float32)
    out += w_o_b[None, None, :]
    if trace:
        return out, res
    return out

